# revision 33
# baseline (speedup 1.0000x reference)
"""Echo State Network kernel for Trainium2 — single-core, full batch.

Math (per reference):
    h_t = tanh(W_in x_t + b_res + W_res h_{t-1}),  h in R^2048, T=1024
    y_t = W_out h_t + b_out

Why one core: each step must stream all of W_res into the PE array
(stationary-operand loads dominate; the moving operand is the batch and
is nearly free up to ~64 columns). 32 sequences on one core cost the
same per step as 4, and single-device dispatch avoids the 8-way
shard_map overhead. The other cores idle.

Design:
  - Fused contraction: W' = [W_res | W_in | b_res | 0] of shape
    [2048, 2176], stored transposed as 17 k-chunks of [128, 2048] in
    SBUF. Each step: 16 output chunks x 17 contraction chunks of
    (LDWEIGHTS + matmul N=32), accumulating into 2 PSUM banks
    (chunks 0-7 / 8-15). ACT applies tanh per half so the next step's
    early k-chunks unblock while the late ones are still activating.
  - All-bf16 weights: microbenchmarks show the (LDWEIGHTS+matmul) pair
    rate is ~34ns regardless of stationary dtype (fp8 made it no
    faster) and regardless of moving width 4..32 - a per-instruction
    floor. So the batch rides free, fp8 would only burn accuracy
    margin, and the step cost is set by instruction count alone.
  - The per-iteration x-block is staged into a statically-addressed
    SBUF tile once per For_i iteration so no matmul carries a
    register-offset (dynamic) access pattern.
  - States live only in an SBUF ring of RING=8 steps; every RING steps
    the readout y = W_out h + b_out runs as 16 matmuls of N=256
    directly from the ring (states never touch DRAM).
  - For_i unrolled by RING so ring slots are static addresses.
"""

from contextlib import ExitStack

import numpy as np
import ml_dtypes

import concourse.bass as bass
import concourse.tile as tile
from concourse import bacc, mybir
from concourse.bass import ds
from concourse.bass_utils import run_bass_kernel_spmd

BF16 = mybir.dt.bfloat16
F32 = mybir.dt.float32
AF = mybir.ActivationFunctionType

N_CORES = 1
B, T_FULL, N_IN, N_RES, N_OUT = 32, 1024, 64, 2048, 64
NCH = N_RES // 128         # 16 output chunks of 128
KCH = NCH + 1              # contraction chunks: 16 reservoir + 1 (x, bias)
KDIM = KCH * 128           # 2176 padded contraction size
RING = 16                  # SBUF state-ring depth = steps per For_i iter
                           # (the For_i back-edge measures ~9.6us/iter:
                           # barrier + IRAM refetch + queue drain, so
                           # amortize it over as many steps as PSUM and
                           # compile time allow)
HB = NCH * B               # 512: one step's state row [128, HB]

LAST_RESULTS = None        # BassKernelResults of the most recent run (for test.py)


def build_module(T=T_FULL, repeat=1, staggered=False):
    nc = bacc.Bacc("TRN2")
    wt = nc.dram_tensor("wt", [KDIM, N_RES], BF16, kind="ExternalInput")
    xb = nc.dram_tensor("xb", [128, T * B], BF16, kind="ExternalInput")
    wo = nc.dram_tensor("wo", [N_RES, N_OUT], BF16, kind="ExternalInput")
    bo = nc.dram_tensor("bo", [N_OUT, 1], F32, kind="ExternalInput")
    # y ships bf16: per-call cost scales with output bytes (~0.4ms/MB
    # through the axon proxy), so halving the 8MB f32 output saves
    # ~1.5ms/call; the host casts back to f32. Costs ~0.2% extra error.
    y = nc.dram_tensor("y", [N_OUT, T * B], BF16, kind="ExternalOutput")

    with tile.TileContext(nc) as tc, ExitStack() as ctx:
        singles = ctx.enter_context(tc.tile_pool(name="singles", bufs=1))
        psum_pool = ctx.enter_context(
            tc.tile_pool(name="psum", bufs=2, space="PSUM")
        )
        ypsum_pool = ctx.enter_context(
            tc.tile_pool(name="ypsum", bufs=2, space="PSUM")
        )
        ysb_pool = ctx.enter_context(tc.tile_pool(name="ysb", bufs=2))

        w_sb = singles.tile([128, KCH * N_RES], BF16)
        nc.sync.dma_start(
            w_sb[:].rearrange("p (j n) -> p j n", n=N_RES),
            wt.rearrange("(j p) n -> p j n", p=128),
        )
        xb_sb = singles.tile([128, T * B], BF16)
        nc.sync.dma_start(xb_sb[:], xb[:, :])
        wo_sb = singles.tile([128, NCH * N_OUT], BF16)
        nc.sync.dma_start(
            wo_sb[:].rearrange("p (k o) -> p k o", o=N_OUT),
            wo.rearrange("(k p) o -> p k o", p=128),
        )
        bo_sb = singles.tile([N_OUT, 1], F32)
        nc.sync.dma_start(bo_sb[:], bo[:, :])

        # State ring, reservoir-major: slot s, chunk j, batch b at
        # Hring[p, 512*s + 32*j + b] = h[128*j + p, b] of step t=s (mod RING).
        Hring = singles.tile([128, RING * HB], BF16)
        Hr3 = Hring[:].rearrange("p (s f) -> p s f", f=HB)

        def w_tile(j, i):
            base = N_RES * j + 128 * i
            return w_sb[:, base : base + 128]

        # Static staging tile for this iteration's RING timesteps of x,
        # so every matmul uses compile-time SBUF addresses.
        xstage = singles.tile([128, RING * B], BF16)

        def step(s):
            # MMs for output chunks 0-7 -> ps_a (one PSUM bank),
            # 8-15 -> ps_b. Two phases: first every group's x-chunk and
            # early k-chunks (0..7, which the previous step's first tanh
            # produced), then every group's late k-chunks (8..15). This
            # gives the PE ~16x9 matmuls of ready work while the previous
            # step's second tanh half is still in flight.
            ps_a = psum_pool.tile([128, 8 * B], F32, tag="psa")
            ps_b = psum_pool.tile([128, 8 * B], F32, tag="psb")
            xcol = xstage[:, s * B : (s + 1) * B]
            src = Hring[:, (s - 1) % RING * HB : ((s - 1) % RING + 1) * HB]

            def out_ap(i):
                ps = ps_a if i < 8 else ps_b
                return ps[:, B * (i % 8) : B * (i % 8 + 1)]

            # start=True clears has_written for the WHOLE bank, so it may
            # appear exactly once per bank per step: on the first matmul
            # into that bank (groups 0 and 8). Every other matmul relies
            # on per-element has_written (first touch overwrites, later
            # touches accumulate). The x-chunk goes last in phase A so
            # the iteration's xstage copy has time to land.
            for i in range(NCH):
                out = out_ap(i)
                for j in range(NCH // 2):
                    nc.tensor.matmul(
                        out,
                        w_tile(j, i),
                        src[:, B * j : B * (j + 1)],
                        start=(i % 8 == 0 and j == 0),
                        stop=False,
                        skip_group_check=True,
                    )
                nc.tensor.matmul(
                    out,
                    w_tile(KCH - 1, i),
                    xcol,
                    start=False,
                    stop=False,
                    skip_group_check=True,
                )
            for i in range(NCH):
                out = out_ap(i)
                for j in range(NCH // 2, NCH):
                    nc.tensor.matmul(
                        out,
                        w_tile(j, i),
                        src[:, B * j : B * (j + 1)],
                        start=False,
                        stop=(j == NCH - 1),
                        skip_group_check=True,
                    )
            dst = Hring[:, s * HB : (s + 1) * HB]
            nc.scalar.activation(dst[:, : 8 * B], ps_a[:], AF.Tanh)
            nc.scalar.activation(dst[:, 8 * B :], ps_b[:], AF.Tanh)

        for _rep in range(repeat):
            # h_{-1} = 0 lives in slot RING-1 (read by step 0).
            nc.vector.memset(Hring[:, (RING - 1) * HB : RING * HB], 0.0)
            with tc.For_i(
                0,
                T,
                RING,
                hint_engines=(mybir.EngineType.PE,),
                staggered_reset=staggered,
            ) as iv:
                nc.vector.tensor_copy(
                    xstage[:], xb_sb[:, ds(iv * B, RING * B)]
                )
                for s in range(RING):
                    step(s)
                # Readout for the RING steps just produced:
                # y[o, (t, b)] = sum_k W_out[o, 128k+p] h[128k+p, (t, b)]
                yp = ypsum_pool.tile([N_OUT, RING * B], F32, tag="yp")
                for k in range(NCH):
                    nc.tensor.matmul(
                        yp[:],
                        wo_sb[:, N_OUT * k : N_OUT * (k + 1)],
                        Hr3[:, :, B * k : B * (k + 1)],
                        start=(k == 0),
                        stop=(k == NCH - 1),
                    )
                ysb = ysb_pool.tile([N_OUT, RING * B], BF16, tag="ysb")
                nc.vector.tensor_scalar_add(ysb[:], yp[:], bo_sb[:, 0:1])
                nc.sync.dma_start(y[:, ds(iv * B, RING * B)], ysb[:])

    nc.finalize()
    return nc


def prep_inputs(x, W_in, W_res, b_res, W_out, b_out, T=T_FULL):
    bf = ml_dtypes.bfloat16
    Wp = np.concatenate(
        [
            W_res,
            W_in,
            b_res[:, None],
            np.zeros((N_RES, KDIM - N_RES - N_IN - 1), np.float32),
        ],
        axis=1,
    )
    wt = np.ascontiguousarray(Wp.T).astype(bf)            # [2176, 2048]
    wo = np.ascontiguousarray(W_out.T).astype(bf)         # [2048, 64]
    bo = np.ascontiguousarray(b_out[:, None]).astype(np.float32)
    xs = x[:, :T]                                         # [B, T, N_IN]
    xbc = np.zeros((128, T * B), bf)
    xbc[:N_IN] = (
        np.ascontiguousarray(xs.transpose(2, 1, 0).reshape(N_IN, T * B))
        .astype(bf)
    )
    xbc[N_IN] = bf(1.0)
    return [{"wt": wt, "xb": xbc, "wo": wo, "bo": bo}]


def assemble_output(results, T=T_FULL):
    yc = results[0]["y"].astype(np.float32)               # [64, T*B]
    return np.ascontiguousarray(
        yc.reshape(N_OUT, T, B).transpose(2, 1, 0)
    )


def run(x, W_in, W_res, b_res, W_out, b_out, T=T_FULL, **run_kwargs):
    global LAST_RESULTS
    in_maps = prep_inputs(x, W_in, W_res, b_res, W_out, b_out, T=T)
    nc = build_module(T=T)
    res = run_bass_kernel_spmd(
        nc, in_maps, core_ids=list(range(N_CORES)), **run_kwargs
    )
    LAST_RESULTS = res
    return assemble_output(res.results, T=T)


def kernel(x, W_in, W_res, b_res, W_out, b_out):
    return run(
        np.asarray(x, np.float32),
        np.asarray(W_in, np.float32),
        np.asarray(W_res, np.float32),
        np.asarray(b_res, np.float32),
        np.asarray(W_out, np.float32),
        np.asarray(b_out, np.float32),
    )



# revision 34
# speedup vs baseline: 1.3456x; 1.3456x over previous
"""Echo State Network kernel for Trainium2 — single-core, full batch.

Math (per reference):
    h_t = tanh(W_in x_t + b_res + W_res h_{t-1}),  h in R^2048, T=1024
    y_t = W_out h_t + b_out

Why one core: each step must stream all of W_res into the PE array
(stationary-operand loads dominate; the moving operand is the batch and
is nearly free up to ~64 columns). 32 sequences on one core cost the
same per step as 4, and single-device dispatch avoids the 8-way
shard_map overhead. The other cores idle.

Design:
  - Fused contraction: W' = [W_res | W_in | b_res | 0] of shape
    [2048, 2176], stored transposed as 17 k-chunks of [128, 2048] in
    SBUF. Each step: 16 output chunks x 17 contraction chunks of
    (LDWEIGHTS + matmul N=32), accumulating into 2 PSUM banks
    (chunks 0-7 / 8-15). ACT applies tanh per half so the next step's
    early k-chunks unblock while the late ones are still activating.
  - All-bf16 weights: microbenchmarks show the (LDWEIGHTS+matmul) pair
    rate is ~34ns regardless of stationary dtype (fp8 made it no
    faster) and regardless of moving width 4..32 - a per-instruction
    floor. So the batch rides free, fp8 would only burn accuracy
    margin, and the step cost is set by instruction count alone.
  - The per-iteration x-block is staged into a statically-addressed
    SBUF tile once per For_i iteration so no matmul carries a
    register-offset (dynamic) access pattern.
  - States live only in an SBUF ring of RING steps; every RING steps
    the readout y = W_out h + b_out runs as 16 matmuls of N=RING*32
    directly from the ring (states never touch DRAM).
  - For_i unrolled by RING so ring slots are static addresses.
"""

from contextlib import ExitStack

import numpy as np
import ml_dtypes

import concourse.bass as bass
import concourse.tile as tile
from concourse import bacc, mybir
from concourse.bass import ds
from concourse.bass_utils import run_bass_kernel_spmd

BF16 = mybir.dt.bfloat16
F32 = mybir.dt.float32
AF = mybir.ActivationFunctionType

N_CORES = 1
B, T_FULL, N_IN, N_RES, N_OUT = 32, 1024, 64, 2048, 64
NCH = N_RES // 128         # 16 output chunks of 128
KCH = NCH + 1              # contraction chunks: 16 reservoir + 1 (x, bias)
KDIM = KCH * 128           # 2176 padded contraction size
RING = 16                  # SBUF state-ring depth = steps per For_i iter
                           # (the For_i back-edge measures ~9.6us/iter:
                           # barrier + IRAM refetch + queue drain, so
                           # amortize it over as many steps as PSUM and
                           # compile time allow)
HB = NCH * B               # 512: one step's state row [128, HB]

LAST_RESULTS = None        # BassKernelResults of the most recent run (for test.py)


def build_module(T=T_FULL, repeat=1, staggered=False):
    nc = bacc.Bacc("TRN2")
    wt = nc.dram_tensor("wt", [KDIM, N_RES], BF16, kind="ExternalInput")
    xb = nc.dram_tensor("xb", [128, T * B], BF16, kind="ExternalInput")
    wo = nc.dram_tensor("wo", [N_RES, N_OUT], BF16, kind="ExternalInput")
    bo = nc.dram_tensor("bo", [N_OUT, 1], F32, kind="ExternalInput")
    # y ships bf16: per-call cost scales with output bytes (~0.4ms/MB
    # through the axon proxy), so halving the 8MB f32 output saves
    # ~1.5ms/call; the host casts back to f32. Costs ~0.2% extra error.
    y = nc.dram_tensor("y", [N_OUT, T * B], BF16, kind="ExternalOutput")

    with tile.TileContext(nc) as tc, ExitStack() as ctx:
        singles = ctx.enter_context(tc.tile_pool(name="singles", bufs=1))
        psum_pool = ctx.enter_context(
            tc.tile_pool(name="psum", bufs=2, space="PSUM")
        )
        ypsum_pool = ctx.enter_context(
            tc.tile_pool(name="ypsum", bufs=2, space="PSUM")
        )
        ysb_pool = ctx.enter_context(tc.tile_pool(name="ysb", bufs=2))

        w_sb = singles.tile([128, KCH * N_RES], BF16)
        nc.sync.dma_start(
            w_sb[:].rearrange("p (j n) -> p j n", n=N_RES),
            wt.rearrange("(j p) n -> p j n", p=128),
        )
        xb_sb = singles.tile([128, T * B], BF16)
        nc.sync.dma_start(xb_sb[:], xb[:, :])
        wo_sb = singles.tile([128, NCH * N_OUT], BF16)
        nc.sync.dma_start(
            wo_sb[:].rearrange("p (k o) -> p k o", o=N_OUT),
            wo.rearrange("(k p) o -> p k o", p=128),
        )
        bo_sb = singles.tile([N_OUT, 1], F32)
        nc.sync.dma_start(bo_sb[:], bo[:, :])

        # State ring, reservoir-major: slot s, chunk j, batch b at
        # Hring[p, 512*s + 32*j + b] = h[128*j + p, b] of step t=s (mod RING).
        Hring = singles.tile([128, RING * HB], BF16)
        Hr3 = Hring[:].rearrange("p (s f) -> p s f", f=HB)

        def w_tile(j, i):
            base = N_RES * j + 128 * i
            return w_sb[:, base : base + 128]

        # Static staging tile for this iteration's RING timesteps of x,
        # so every matmul uses compile-time SBUF addresses.
        xstage = singles.tile([128, RING * B], BF16)

        def step(s):
            # MMs for output chunks 0-7 -> ps_a (one PSUM bank),
            # 8-15 -> ps_b. Two phases: first every group's x-chunk and
            # early k-chunks (0..7, which the previous step's first tanh
            # produced), then every group's late k-chunks (8..15). This
            # gives the PE ~16x9 matmuls of ready work while the previous
            # step's second tanh half is still in flight.
            ps_a = psum_pool.tile([128, 8 * B], F32, tag="psa")
            ps_b = psum_pool.tile([128, 8 * B], F32, tag="psb")
            xcol = xstage[:, s * B : (s + 1) * B]
            src = Hring[:, (s - 1) % RING * HB : ((s - 1) % RING + 1) * HB]

            def out_ap(i):
                ps = ps_a if i < 8 else ps_b
                return ps[:, B * (i % 8) : B * (i % 8 + 1)]

            # start=True clears has_written for the WHOLE bank, so it may
            # appear exactly once per bank per step: on the first matmul
            # into that bank (groups 0 and 8). Every other matmul relies
            # on per-element has_written (first touch overwrites, later
            # touches accumulate). The x-chunk goes last in phase A so
            # the iteration's xstage copy has time to land.
            for i in range(NCH):
                out = out_ap(i)
                for j in range(NCH // 2):
                    nc.tensor.matmul(
                        out,
                        w_tile(j, i),
                        src[:, B * j : B * (j + 1)],
                        start=(i % 8 == 0 and j == 0),
                        stop=False,
                        skip_group_check=True,
                    )
                nc.tensor.matmul(
                    out,
                    w_tile(KCH - 1, i),
                    xcol,
                    start=False,
                    stop=False,
                    skip_group_check=True,
                )
            for i in range(NCH):
                out = out_ap(i)
                for j in range(NCH // 2, NCH):
                    nc.tensor.matmul(
                        out,
                        w_tile(j, i),
                        src[:, B * j : B * (j + 1)],
                        start=False,
                        stop=(j == NCH - 1),
                        skip_group_check=True,
                    )
            dst = Hring[:, s * HB : (s + 1) * HB]
            nc.scalar.activation(dst[:, : 8 * B], ps_a[:], AF.Tanh)
            nc.scalar.activation(dst[:, 8 * B :], ps_b[:], AF.Tanh)

        for _rep in range(repeat):
            # h_{-1} = 0 lives in slot RING-1 (read by step 0).
            nc.vector.memset(Hring[:, (RING - 1) * HB : RING * HB], 0.0)
            with tc.For_i(
                0,
                T,
                RING,
                hint_engines=(mybir.EngineType.PE,),
                staggered_reset=staggered,
            ) as iv:
                nc.vector.tensor_copy(
                    xstage[:], xb_sb[:, ds(iv * B, RING * B)]
                )
                for s in range(RING):
                    step(s)
                # Readout for the RING steps just produced:
                # y[o, (t, b)] = sum_k W_out[o, 128k+p] h[128k+p, (t, b)]
                yp = ypsum_pool.tile([N_OUT, RING * B], F32, tag="yp")
                for k in range(NCH):
                    nc.tensor.matmul(
                        yp[:],
                        wo_sb[:, N_OUT * k : N_OUT * (k + 1)],
                        Hr3[:, :, B * k : B * (k + 1)],
                        start=(k == 0),
                        stop=(k == NCH - 1),
                    )
                ysb = ysb_pool.tile([N_OUT, RING * B], BF16, tag="ysb")
                nc.vector.tensor_scalar_add(ysb[:], yp[:], bo_sb[:, 0:1])
                nc.sync.dma_start(y[:, ds(iv * B, RING * B)], ysb[:])

    nc.finalize()
    return nc


def prep_inputs(x, W_in, W_res, b_res, W_out, b_out, T=T_FULL):
    bf = ml_dtypes.bfloat16
    Wp = np.concatenate(
        [
            W_res,
            W_in,
            b_res[:, None],
            np.zeros((N_RES, KDIM - N_RES - N_IN - 1), np.float32),
        ],
        axis=1,
    )
    wt = np.ascontiguousarray(Wp.T).astype(bf)            # [2176, 2048]
    wo = np.ascontiguousarray(W_out.T).astype(bf)         # [2048, 64]
    bo = np.ascontiguousarray(b_out[:, None]).astype(np.float32)
    xs = x[:, :T]                                         # [B, T, N_IN]
    xbc = np.zeros((128, T * B), bf)
    xbc[:N_IN] = (
        np.ascontiguousarray(xs.transpose(2, 1, 0).reshape(N_IN, T * B))
        .astype(bf)
    )
    xbc[N_IN] = bf(1.0)
    return [{"wt": wt, "xb": xbc, "wo": wo, "bo": bo}]


def assemble_output(results, T=T_FULL):
    yc = results[0]["y"].astype(np.float32)               # [64, T*B]
    return np.ascontiguousarray(
        yc.reshape(N_OUT, T, B).transpose(2, 1, 0)
    )


def run(x, W_in, W_res, b_res, W_out, b_out, T=T_FULL, **run_kwargs):
    global LAST_RESULTS
    in_maps = prep_inputs(x, W_in, W_res, b_res, W_out, b_out, T=T)
    nc = build_module(T=T)
    res = run_bass_kernel_spmd(
        nc, in_maps, core_ids=list(range(N_CORES)), **run_kwargs
    )
    LAST_RESULTS = res
    return assemble_output(res.results, T=T)


def kernel(x, W_in, W_res, b_res, W_out, b_out):
    return run(
        np.asarray(x, np.float32),
        np.asarray(W_in, np.float32),
        np.asarray(W_res, np.float32),
        np.asarray(b_res, np.float32),
        np.asarray(W_out, np.float32),
        np.asarray(b_out, np.float32),
    )

